# revision 38
# baseline (speedup 1.0000x reference)
"""Trainium2 Bass kernel for nn_DecomposedAttention (B=2,H=8,N=2048,D=64).

Algebra: the reference chain
    Qt  = Q^T
    QX  = Q @ Qt                      [N,N]
    KXT = (K @ Qt)^T = Q @ K^T        [N,N]
    VX  = V @ Qt / 64                 [N,N]
    out = QX @ (KXT @ VX)
collapses (every big factor is rank-D) to
    out = Q @ [ (Q^T Q) @ (K^T V) / 64 ] @ Q^T  =  Q @ M @ Q^T,   M: [64,64]
so per (b,h) the work is two 64x64 Gram matrices, a tiny GEMM, and one
[N,64] @ [64,N] outer-product GEMM streamed to HBM.  The kernel is purely
output-bandwidth bound.

Precision/bandwidth: the whole on-device data path runs in fp16.  Inputs are
cast to fp16 on the host during sharding (Q additionally shipped
pre-transposed, same rounding); M is scaled by 2^-16 when cast to fp16 so
W = M Q^T / 2^16 and the output tile Q W stay inside fp16 range (max |out| /
2^16 ~ 42 on this distribution); the host multiplies the returned fp16
output by 2^16 (exact, power of two).  Measured end-to-end rel err ~4e-4.
Versus the fp32/bf16-hi-lo scheme this halves the dominant output DMA
traffic (16.8 MB/core instead of 33.5) and halves PE time in the big GEMM
(one fp16 matmul per output chunk, contraction 64, instead of two bf16
hi/lo matmuls with contraction 128).

Sharding: B*H = 16 head-pairs, 2 per core across 8 cores (pure data
parallelism, no communication).

Layouts: q/k/v are shipped as [128, 1024] fp16 per head (partition p holds
tokens 16p..16p+15 - a pure reshape of the row-major [N, D] array), so the
DMA moves 2 KB contiguous runs per partition (full-rate, >= 512 B).  qt is
[64, 2048] fp16.  Slice [:, 64s:64s+64] of q/k/v is the [128, 64] token
block used by the Gram accumulations.
"""

import numpy as np

import concourse.bass as bass
import concourse.mybir as mybir
from concourse import bacc, masks, tile
from concourse.bass_utils import run_bass_kernel_spmd

B, H, N, D = 2, 8, 2048, 64
SCALE = 64.0
MT_SCALE = 65536.0  # SCALE * 1024; folded into the fp16 cast of M
HOST_SCALE = MT_SCALE / SCALE  # 1024: the range-headroom part, undone on host
N_CORES = 8
HPC = (B * H) // N_CORES  # heads per core = 2
NS = N // 128  # 16 row-tiles / Gram slices per head

F32 = mybir.dt.float32
F16 = mybir.dt.float16

_CACHED = None


def _build_nc():
    nc = bacc.Bacc("TRN2", target_bir_lowering=False, debug=False)

    q = nc.dram_tensor("q", [HPC, 128, NS * D], F16, kind="ExternalInput")
    qt = nc.dram_tensor("qt", [HPC, D, N], F16, kind="ExternalInput")
    k = nc.dram_tensor("k", [HPC, 128, NS * D], F16, kind="ExternalInput")
    v = nc.dram_tensor("v", [HPC, 128, NS * D], F16, kind="ExternalInput")
    o = nc.dram_tensor("o", [HPC, N, N], F16, kind="ExternalOutput")

    with tile.TileContext(nc) as tc:
        with (
            tc.tile_pool(name="const", bufs=1) as constp,
            tc.tile_pool(name="qin", bufs=2) as qinp,
            tc.tile_pool(name="kvin", bufs=2) as kvinp,
            tc.tile_pool(name="qtp", bufs=2) as qtp,
            tc.tile_pool(name="small", bufs=2) as smallp,
            tc.tile_pool(name="stat", bufs=2) as statp,
            tc.tile_pool(name="stage", bufs=8) as stagep,
            tc.tile_pool(name="psmall", bufs=2, space="PSUM") as psmall,
            tc.tile_pool(name="psb", bufs=3, space="PSUM") as psb,
        ):
            ident = constp.tile([128, 128], F32)
            masks.make_identity(nc, ident[:])

            # PE warm-up: keep TensorE busy while the first input DMAs land,
            # so the setup matmuls run ramped-up instead of cold.
            wps = psb.tile([128, 1024], F32, tag="big")
            for _ in range(7):
                nc.tensor.matmul(
                    wps[:32, :32], ident[:, :32], ident[:, :32], start=True, stop=True
                )

            st = {}

            def loads(h):
                qc = qinp.tile([128, NS * D], F16, tag="qc")
                kc = kvinp.tile([128, NS * D], F16, tag="kc")
                vc = kvinp.tile([128, NS * D], F16, tag="vc")
                qts = qtp.tile([64, N], F16, tag="qt")
                # k, v first: C = K^T V is the longest Gram dependency of the
                # M chain; qt last (W also waits on M anyway)
                nc.sync.dma_start(kc[:], k[h])
                nc.sync.dma_start(vc[:], v[h])
                nc.sync.dma_start(qc[:], q[h])
                nc.sync.dma_start(qts[:], qt[h])
                st[h] = dict(qc=qc, kc=kc, vc=vc, qts=qts)

            def setup(h, chunk_cb=None):
                """Generator: per-head preprocessing, yields at cheap
                suspension points so head h+1's setup can interleave with
                head h's big loop.  chunk_cb(c) is invoked right after W
                chunk c's cast is emitted (used to pipeline head 0's first
                output tile into the W chain)."""
                d = st[h]
                qc, kc, vc, qts = d["qc"], d["kc"], d["vc"], d["qts"]

                # C = K^T V (fp16 inputs, fp32 PSUM accumulation)
                c_ps = psmall.tile([64, 64], F32, tag="ps")
                for s in range(NS):
                    sl = slice(64 * s, 64 * (s + 1))
                    nc.tensor.matmul(
                        c_ps[:], kc[:, sl], vc[:, sl],
                        start=(s == 0), stop=(s == NS - 1),
                    )
                    if s % 8 == 7:
                        yield
                c_sb = smallp.tile([64, 64], F16, tag="c")
                nc.scalar.copy(c_sb[:], c_ps[:])

                # A = Q^T Q (on DVE so it doesn't serialize behind c_sb)
                a_ps = psmall.tile([64, 64], F32, tag="ps")
                for s in range(NS):
                    sl = slice(64 * s, 64 * (s + 1))
                    nc.tensor.matmul(
                        a_ps[:], qc[:, sl], qc[:, sl],
                        start=(s == 0), stop=(s == NS - 1),
                    )
                    if s % 8 == 7:
                        yield
                a_sb = smallp.tile([64, 64], F16, tag="a")
                nc.vector.tensor_copy(a_sb[:], a_ps[:])

                # Mt = C^T A = M^T (A symmetric); cast fp16 with 2^-16 folded
                mt_ps = psmall.tile([64, 64], F32, tag="ps")
                nc.tensor.matmul(mt_ps[:], c_sb[:], a_sb[:], start=True, stop=True)
                mt16 = smallp.tile([64, 64], F16, tag="mt")
                nc.vector.tensor_scalar_mul(mt16[:], mt_ps[:], 1.0 / MT_SCALE)
                yield

                # W = (M / 2^16) @ Q^T, fp16 stationary for the big loop.
                # Cast chunk-by-chunk so big tile 0 can start on chunk 0.
                wst = statp.tile([64, N], F16, tag="wst")
                d["wst"] = wst
                for c in range(4):
                    sl = slice(512 * c, 512 * (c + 1))
                    w_ps = psmall.tile([64, 512], F32, tag="ps")
                    nc.tensor.matmul(w_ps[:], mt16[:], qts[:, sl], start=True, stop=True)
                    if c % 2 == 0:
                        nc.scalar.copy(wst[:, sl], w_ps[:])
                    else:
                        nc.vector.tensor_copy(wst[:, sl], w_ps[:])
                    if chunk_cb is not None:
                        chunk_cb(c)
                    yield

            def big_tile(h, t, split_dma=False):
                d = st[h]
                qts, wst = d["qts"], d["wst"]
                stg = stagep.tile([128, N], F16, tag="stage")
                lhs = qts[:, 128 * t : 128 * (t + 1)]
                rows = slice(128 * t, 128 * (t + 1))
                if split_dma == 4:
                    # quarter-granular: cast+DMA each 512-chunk as it lands
                    # (alternating ACT/DVE) to shorten the pipeline tail
                    for half in range(2):
                        pb = psb.tile([128, 1024], F32, tag="big")
                        for j in range(2):
                            c = 2 * half + j
                            cs = slice(512 * c, 512 * (c + 1))
                            ps = pb[:, 512 * j : 512 * (j + 1)]
                            nc.tensor.matmul(ps, lhs, wst[:, cs], start=True, stop=True)
                            if c % 2 == 0:
                                nc.scalar.copy(stg[:, cs], ps)
                            else:
                                nc.vector.tensor_copy(stg[:, cs], ps)
                            nc.sync.dma_start(o[h, rows, cs], stg[:, cs])
                    return
                pb0 = psb.tile([128, 1024], F32, tag="big")
                nc.tensor.matmul(pb0[:, 0:512], lhs, wst[:, 0:512], start=True, stop=True)
                nc.tensor.matmul(pb0[:, 512:1024], lhs, wst[:, 512:1024], start=True, stop=True)
                nc.scalar.copy(stg[:, 0:1024], pb0[:])
                if split_dma:
                    nc.sync.dma_start(o[h, rows, 0:1024], stg[:, 0:1024])
                pb1 = psb.tile([128, 1024], F32, tag="big")
                nc.tensor.matmul(pb1[:, 0:512], lhs, wst[:, 1024:1536], start=True, stop=True)
                nc.tensor.matmul(pb1[:, 512:1024], lhs, wst[:, 1536:2048], start=True, stop=True)
                nc.vector.tensor_copy(stg[:, 1024:2048], pb1[:])
                if split_dma:
                    nc.sync.dma_start(o[h, rows, 1024:2048], stg[:, 1024:2048])
                else:
                    nc.sync.dma_start(o[h, rows, :], stg[:])

            def drain(gen):
                if gen is not None:
                    for _ in gen:
                        pass

            def emit_all():
                loads(0)
                loads(1)

                # Ramp: W chunks 0-1 unlock the first half (cols 0:1024) of
                # EVERY tile, so a-halves of t0..t3 are emitted inside the W
                # chain (alternating cast engines) and their b-halves right
                # after W chunks 2-3 - both cast engines produce back-to-back
                # from the first output byte on.
                t0_stg = {}

                def half_tile(t, half, eng, split=False):
                    d = st[0]
                    qts, wst = d["qts"], d["wst"]
                    if half == 0:
                        t0_stg[t] = stagep.tile(
                            [128, N], F16, tag="stage", name=f"stg_r{t}"
                        )
                    stg = t0_stg[t]
                    lhs = qts[:, 128 * t : 128 * (t + 1)]
                    rows = slice(128 * t, 128 * (t + 1))
                    pb = psb.tile([128, 1024], F32, tag="big", name=f"pb_r{t}_{half}")
                    hs = slice(1024 * half, 1024 * half + 1024)
                    nc.tensor.matmul(
                        pb[:, 0:512], lhs, wst[:, 1024 * half : 1024 * half + 512],
                        start=True, stop=True,
                    )
                    if split:
                        # quarter-granular cast+DMA on the same engine: the
                        # first piece's DMA-issue latency hides behind the
                        # second piece's cast
                        q0 = slice(1024 * half, 1024 * half + 512)
                        q1 = slice(1024 * half + 512, 1024 * (half + 1))
                        cp = nc.scalar.copy if eng == "act" else nc.vector.tensor_copy
                        cp(stg[:, q0], pb[:, 0:512])
                        nc.sync.dma_start(o[0, rows, q0], stg[:, q0])
                        nc.tensor.matmul(
                            pb[:, 512:1024], lhs,
                            wst[:, 1024 * half + 512 : 1024 * (half + 1)],
                            start=True, stop=True,
                        )
                        cp(stg[:, q1], pb[:, 512:1024])
                        nc.sync.dma_start(o[0, rows, q1], stg[:, q1])
                        return
                    nc.tensor.matmul(
                        pb[:, 512:1024], lhs, wst[:, 1024 * half + 512 : 1024 * (half + 1)],
                        start=True, stop=True,
                    )
                    if eng == "act":
                        nc.scalar.copy(stg[:, hs], pb[:])
                    else:
                        nc.vector.tensor_copy(stg[:, hs], pb[:])
                    nc.sync.dma_start(o[0, rows, hs], stg[:, hs])

                def ramp_cb(c):
                    if c == 1:
                        # t0's first half in two 512-col pieces, BOTH on ACT
                        # (piece 0 needs only W chunk 0 -> earliest possible
                        # first output DMA; keeping DVE free preserves the W1
                        # cast timing that everything else hangs off)
                        d = st[0]
                        qts, wst = d["qts"], d["wst"]
                        t0_stg[0] = stagep.tile(
                            [128, N], F16, tag="stage", name="stg_r0"
                        )
                        stg = t0_stg[0]
                        lhs = qts[:, 0:128]
                        pb = psb.tile([128, 1024], F32, tag="big", name="pb_r0_0")
                        nc.tensor.matmul(pb[:, 0:512], lhs, wst[:, 0:512], start=True, stop=True)
                        nc.scalar.copy(stg[:, 0:512], pb[:, 0:512])
                        nc.sync.dma_start(o[0, 0:128, 0:512], stg[:, 0:512])
                        nc.tensor.matmul(pb[:, 512:1024], lhs, wst[:, 512:1024], start=True, stop=True)
                        nc.scalar.copy(stg[:, 512:1024], pb[:, 512:1024])
                        nc.sync.dma_start(o[0, 0:128, 512:1024], stg[:, 512:1024])
                        half_tile(1, 0, "dve")
                    elif c == 2:
                        half_tile(2, 0, "act")
                    elif c == 3:
                        half_tile(3, 0, "dve")
                        half_tile(0, 1, "act")
                        half_tile(1, 1, "dve")
                        half_tile(2, 1, "act")
                        half_tile(3, 1, "dve")

                drain(setup(0, chunk_cb=ramp_cb))
                nxt = setup(1)
                for t in range(4, NS):
                    big_tile(0, t)
                    if t < 5:
                        # keep head-1 setup out of the engine queues while
                        # head 0's ramp is still latency-critical
                        continue
                    for _ in range(2):
                        if nxt is not None and (
                            next(nxt, StopIteration) is StopIteration
                        ):
                            nxt = None
                drain(nxt)
                for t in range(NS):
                    big_tile(1, t, split_dma=(4 if t == NS - 1 else t == NS - 2))

            emit_all()

    nc.compile()
    return nc


def _get_nc():
    global _CACHED
    if _CACHED is None:
        _CACHED = _build_nc()
    return _CACHED


def _run(Q, K, V, **spmd_kwargs):
    BH = B * H
    q16 = np.asarray(Q, dtype=np.float32).reshape(BH, N, D).astype(np.float16)
    k16 = np.asarray(K, dtype=np.float32).reshape(BH, N, D).astype(np.float16)
    v16 = np.asarray(V, dtype=np.float32).reshape(BH, N, D).astype(np.float16)
    # partition p holds tokens 16p..16p+15: a pure reshape of row-major [N,D]
    qr = np.ascontiguousarray(q16.reshape(BH, 128, NS * D))
    kr = np.ascontiguousarray(k16.reshape(BH, 128, NS * D))
    vr = np.ascontiguousarray(v16.reshape(BH, 128, NS * D))
    # same rounding as qr (transpose of the already-rounded fp16 array)
    qtr = np.ascontiguousarray(np.swapaxes(q16, 1, 2))

    nc = _get_nc()
    in_maps = [
        {
            "q": qr[c * HPC : (c + 1) * HPC],
            "qt": qtr[c * HPC : (c + 1) * HPC],
            "k": kr[c * HPC : (c + 1) * HPC],
            "v": vr[c * HPC : (c + 1) * HPC],
        }
        for c in range(N_CORES)
    ]
    res = run_bass_kernel_spmd(
        nc, in_maps, core_ids=list(range(N_CORES)), **spmd_kwargs
    )
    out = np.concatenate(
        [np.asarray(res.results[c]["o"]) for c in range(N_CORES)], axis=0
    )
    out = out.astype(np.float32) * np.float32(HOST_SCALE)
    return out.reshape(B, H, N, N), res


def kernel(X=None, Q=None, K=None, V=None):
    out, _ = _run(Q, K, V)
    return out
